# revision 10
# baseline (speedup 1.0000x reference)
"""LSTM caption decoder on 8 TRN2 NeuronCores — hidden-sharded, SBUF->SBUF
remote-DMA h exchange (replaces the ncfw AllGather of the baseline).

Per step, each core broadcasts its h_t chunk [128, B] bf16 to its 7
XOR-partners with 7 single-destination remote_dma_broadcasts: the send to
XOR-distance d lands in the receiver's ring slot d. Receiver slot s therefore
always holds the chunk of its XOR-s partner; the per-core whh/wfc/ht0 inputs
are slot-permuted on the host (PERM table, probe-measured; re-verified at
runtime via a topo output, with automatic in_map rebuild + rerun on
mismatch).

Flow control needs no credits: only the PE reads the ring, the PE queue is
in-order, and trigger_t is gated (via Tile's deferred-RAW edge) on the h_t
vector mul, which transitively implies every ring reader of generation t-2
and earlier has retired. With ring depth 3, a peer's sends for generation
t+3 (which require our h_{t+2} chunk, hence our trigger_{t+2}) can therefore
never overwrite a generation still being read.

Cross-core waits (rsem arrivals, lsem drain) cannot be satisfied in Tile's
single-core scheduling sim, so they ride on placeholder wait instructions
(trivially true at scheduling time) whose real condition is added after the
TileContext exits. A placeholder emitted before its consumers on the same
engine queue is scheduled before them (it is always ready and has an earlier
priority), so it gates the whole queue at exactly the right point.
"""

import sys

if "/opt/trn_rl_repo" not in sys.path:
    sys.path.insert(0, "/opt/trn_rl_repo")

import numpy as np
import ml_dtypes

import concourse.bass as bass
import concourse.bacc as bacc
import concourse.tile as tile
from concourse.tile_rust import add_dep_helper
from concourse import mybir
from concourse.bass_utils import run_bass_kernel_spmd

B = 128
T = 24
E = 512
H = 1024
V = 12000
NCORES = 8
VS = V // NCORES          # 1500 vocab cols per core
KX = E // 128             # 4 contraction chunks for x-part
KH = H // 128             # 8 contraction chunks for h-part
NVC = 3                   # vocab chunks per core (3 x 500)
VC = VS // NVC            # 500
NSLOT = 3                 # ring depth

F32 = mybir.dt.float32
BF16 = mybir.dt.bfloat16
AF = mybir.ActivationFunctionType

# gate col-chunk order [i, g, f, o] -> activation per chunk
_CC_FUNC = [AF.Sigmoid, AF.Tanh, AF.Sigmoid, AF.Sigmoid]

# PERM[i][s] = jax index of core i's XOR-distance-s partner (probe-measured).
PERM = [
    [0, 1, 2, 3, 6, 7, 4, 5],
    [1, 0, 3, 2, 7, 6, 5, 4],
    [2, 3, 0, 1, 4, 5, 6, 7],
    [3, 2, 1, 0, 5, 4, 7, 6],
    [4, 5, 6, 7, 2, 3, 0, 1],
    [5, 4, 7, 6, 3, 2, 1, 0],
    [6, 7, 4, 5, 0, 1, 2, 3],
    [7, 6, 5, 4, 1, 0, 3, 2],
]

NSTEPS = T - 1
# An ncfw collective at program START crashes the exec unit when the program
# also uses SWDGE remote_dma (this was the earlier session's "repeated
# broadcasts crash"). But a collective-free NEFF loses the runtime's
# load-time rendezvous, so core starts stagger by milliseconds, which lands
# in the profiled span. Solution: one collective at program END, after the
# SWDGE drain.
WARMUP_CC = True


def build_nc(for_sim: bool = False, nsteps: int = NSTEPS):
    nc = bacc.Bacc("TRN2", target_bir_lowering=False, debug=False,
                   num_devices=NCORES)

    wih_d = nc.dram_tensor("wih", [128, KX, 4, 128], BF16, kind="ExternalInput").ap()
    whh_d = nc.dram_tensor("whh", [128, KH, 4, 128], BF16, kind="ExternalInput").ap()
    bg_d = nc.dram_tensor("bg", [128, 4], F32, kind="ExternalInput").ap()
    wfc_d = nc.dram_tensor("wfc", [128, KH, VS], BF16, kind="ExternalInput").ap()
    bfc_d = nc.dram_tensor("bfc", [128, VS], F32, kind="ExternalInput").ap()
    xt_d = nc.dram_tensor("xt", [T - 1, 128, KX, B], BF16, kind="ExternalInput").ap()
    ht0_d = nc.dram_tensor("ht0", [128, KH, B], BF16, kind="ExternalInput").ap()
    c0_d = nc.dram_tensor("c0", [128, B], F32, kind="ExternalInput").ap()
    ids_d = nc.dram_tensor("ids", [128, 16], BF16, kind="ExternalInput").ap()
    out_d = nc.dram_tensor("logits", [T, B, VS], BF16, kind="ExternalOutput").ap()
    topo_d = nc.dram_tensor("topo", [128, 8, 16], BF16, kind="ExternalOutput").ap()
    warm_in_d = nc.dram_tensor("warm_in", [1, 4], BF16, kind="Internal").ap()
    warm_out_d = nc.dram_tensor("warm_out", [KH, 4], BF16, kind="Internal",
                                addr_space="Shared").ap()

    # (placeholder wait instruction, sem, value): real condition added after
    # Tile scheduling — the scheduler's single-core sim can't satisfy
    # cross-core sems and would deadlock.
    pending = []

    import contextlib
    raw = contextlib.ExitStack()
    # ring + topo receive buffers are RAW sbuf tensors, invisible to Tile's
    # dependency tracking: remote writes + the local slot-0 h store are
    # ordered purely by the rsem/hsem protocol. (Tile's conservative WAR
    # edges on tracked tiles serialized the broadcast preps behind late PE
    # ticks, delaying straggler transfers by a whole step.)
    ring = [raw.enter_context(
        nc.sbuf_tensor(f"ring{r}", [128, KH, B], BF16)) for r in range(NSLOT)]
    topo_rx = raw.enter_context(nc.sbuf_tensor("topo_rx", [128, 8, 16], BF16))

    with raw, tile.TileContext(nc) as tc:
        with (
            tc.tile_pool(name="weights", bufs=1) as wpool,
            tc.tile_pool(name="xin", bufs=3) as xpool,
            tc.tile_pool(name="gact", bufs=2) as gpool,
            tc.tile_pool(name="state", bufs=1) as spool,
            tc.tile_pool(name="hbuf", bufs=1) as hpool,
            tc.tile_pool(name="lout", bufs=4) as lpool,
            tc.tile_pool(name="pg", bufs=4, space="PSUM") as pgpool,
            tc.tile_pool(name="pf", bufs=3, space="PSUM") as pfpool,
        ):
            wih = wpool.tile([128, KX, 4, 128], BF16)
            whh = wpool.tile([128, KH, 4, 128], BF16)
            bg = wpool.tile([128, 4], F32)
            wfc = wpool.tile([128, KH, VS], BF16)
            bfc = wpool.tile([128, VS], F32)
            c = spool.tile([128, B], F32)
            tanh_c = spool.tile([128, B], F32)
            ig = spool.tile([128, B], F32)
            ids_t = hpool.tile([128, 16], BF16)

            rsems = [nc.alloc_semaphore(f"rsem{d}") for d in range(8)]
            # arrivals per round: same-die slots (1-3) +16, cross-die +8
            RINC = [0, 16, 16, 16, 8, 8, 8, 8]
            lsem = nc.alloc_semaphore("lsem")     # local send completions
            hsem = nc.alloc_semaphore("hsem")     # h-ready chain (hm_t -> t)
            phsem = nc.alloc_semaphore("phsem")   # placeholder-only, stays 0

            def gate(eng, sem, val, after=()):
                """Placeholder wait on `eng`'s queue; the real cross-core
                condition (sem >= val) is added after Tile scheduling.
                `after`: instructions the gate must schedule behind (no-sync
                edges); callers add gate->consumer edges via before()."""
                ph = eng.wait_ge(phsem, 0)
                for a in after:
                    add_dep_helper(ph.ins, a.ins, sync=False,
                                   reason="gate-after")
                pending.append((ph, sem, val))
                return ph

            def before(ph, consumers):
                """Pin consumers behind the gate in the schedule."""
                if ph is None:
                    return
                for cns in consumers:
                    add_dep_helper(cns.ins, ph.ins, sync=False,
                                   reason="gate-before")


            # prologue loads
            ht0_t = hpool.tile([128, KH, B], BF16, name="ht0t")
            nc.sync.dma_start(ht0_t[:], ht0_d[:])
            nc.sync.dma_start(c[:], c0_d[:])
            xt1 = xpool.tile([128, KX, B], BF16, tag="xt")
            nc.sync.dma_start(xt1[:], xt_d[0])
            nc.sync.dma_start(wih[:], wih_d[:])
            nc.sync.dma_start(bg[:], bg_d[:])
            nc.sync.dma_start(whh[:], whh_d[:])
            nc.sync.dma_start(ids_t[:], ids_d[:])
            nc.sync.dma_start(topo_rx[:, 0, :], ids_d[:, 0:16])
            for v in range(NVC):
                vsl = slice(v * VC, (v + 1) * VC)
                eng = nc.scalar if v == 1 else nc.sync
                eng.dma_start(wfc[:, :, vsl], wfc_d[:, :, vsl])
            nc.scalar.dma_start(bfc[:], bfc_d[:])

            def send_group(src_ap, dst_tile):
                """7 single-dest broadcasts src->slot d + trigger. The one
                destination is repeated across slots so the payload stripes
                over all eligible lanes (16 same-die, 8 D2D for cross-die)
                instead of 2 — the frame is 1024 descriptors either way and
                payload descriptors drain much faster than dummies.
                (Genuinely multi-dest data broadcasts crash the exec unit
                on this runtime.) Each distance gets its own arrival sem so
                receivers can start consuming a slot as soon as it lands."""
                for d in range(1, 8):
                    if d < 4:
                        rd = [(0, d)] * 8
                    else:
                        rd = [None] * 4 + [(0, d)] * 4
                    nc.gpsimd.remote_dma_broadcast(
                        dst_tile[:, d, :], src_ap, rsems[d], lsem, rdests=rd)
                return nc.gpsimd.trigger_dma(count=None)

            # topo round: exchange jax ids (verifies PERM at runtime)
            ttrig = send_group(ids_t[:], topo_rx)
            tph = ttrig
            for d in range(1, 8):
                tph = gate(nc.gpsimd, rsems[d], RINC[d], after=[tph])
            before(tph, [nc.gpsimd.dma_start(topo_d[:], topo_rx[:])])

            def emit_fc(t, rp, ph):
                """FC over ring rp (gen t), pinned behind gate `ph` (the
                same-ring-generation arrival gate)."""
                firsts, last = [], None
                for v in range(NVC):
                    vsl = slice(v * VC, (v + 1) * VC)
                    pf = pfpool.tile([B, VC], F32, tag="pf")
                    for k in range(KH):
                        mm = nc.tensor.matmul(
                            pf[:], rp[:, k, :], wfc[:, k, vsl],
                            start=(k == 0), stop=(k == KH - 1),
                        )
                        if k == 0:
                            firsts.append(mm)
                        last = mm
                    lo = lpool.tile([B, VC], BF16, tag="lo")
                    nc.vector.tensor_add(lo[:], pf[:], bfc[:, vsl])
                    nc.scalar.dma_start(out_d[t, :, vsl], lo[:])
                before(ph, firsts)
                return last

            for t in range(1, nsteps + 1):
                if t == 1:
                    xt = xt1
                else:
                    xt = xpool.tile([128, KX, B], BF16, tag="xt")
                    nc.scalar.dma_start(xt[:], xt_d[t - 1])
                # gen 0 lives in the tracked ht0 tile (Tile orders its DMA);
                # gens >= 1 live in the raw rings
                rp = ht0_t if t == 1 else ring[(t - 1) % NSLOT]
                rc = ring[t % NSLOT]         # will hold h_t

                # ---- gates (transposed): x-parts first, then h-parts ----
                pgs = []
                xp_last = []
                for cc in range(4):
                    pg = pgpool.tile([128, 512], F32, tag="pg")
                    pgs.append(pg)
                    for k in range(KX):
                        mm = nc.tensor.matmul(
                            pg[:, 0:B], wih[:, k, cc, :], xt[:, k, :],
                            start=(k == 0), stop=False,
                        )
                    xp_last.append(mm)
                # slot-major h-part matmuls: slot s's 4 gate-col MMs run as
                # soon as its chunk lands (per-slot gates chained on the PE
                # queue). Slot 0 (own chunk, t>=2) gates on the hsem chain.
                ph = None
                if t >= 2:
                    ph = nc.tensor.wait_ge(hsem, t - 1)
                    for a_ in xp_last:
                        add_dep_helper(ph.ins, a_.ins, sync=False,
                                       reason="gate-after")
                for s in range(KH):
                    if t >= 2 and s >= 1:
                        ph = gate(nc.tensor, rsems[s], RINC[s] * t,
                                  after=[ph] if ph is not None else [])
                    for cc in range(4):
                        mm = nc.tensor.matmul(
                            pgs[cc][:, 0:B], whh[:, s, cc, :], rp[:, s, :],
                            start=False, stop=(s == KH - 1),
                        )
                        before(ph, [mm])
                gact = []
                for cc in range(4):
                    a = gpool.tile([128, B], F32, tag=f"g{cc}")
                    nc.scalar.activation(a[:], pgs[cc][:, 0:B], _CC_FUNC[cc],
                                         bias=bg[:, cc:cc + 1])
                    gact.append(a)
                    if cc == 1:
                        nc.vector.tensor_mul(ig[:], gact[0][:], gact[1][:])
                    elif cc == 2:
                        nc.vector.tensor_mul(c[:], c[:], gact[2][:])
                        nc.vector.tensor_add(c[:], c[:], ig[:])
                        nc.scalar.activation(tanh_c[:], c[:], AF.Tanh)
                    elif cc == 3:
                        hm = nc.vector.tensor_mul(rc[:, 0, :], gact[3][:],
                                                  tanh_c[:])
                        # dedicated sem op (DVE TT has no free update slot);
                        # in-order DVE queue + no-sync edge pins it after hm
                        hs = nc.vector.sem_inc(hsem, 1)
                        add_dep_helper(hs.ins, hm.ins, sync=False,
                                       reason="hsem-after-hm")

                # ---- broadcast h_t (trigger waits h_t via hsem) ----
                trig = send_group(rc[:, 0, :], rc)
                trig._wait_ge(hsem, t)

                # ---- FC of previous step (overlaps the broadcast) ----
                fc_last = emit_fc(t - 1, rp, ph)


            eph = nc.tensor.wait_ge(hsem, nsteps)
            add_dep_helper(eph.ins, fc_last.ins, sync=False,
                           reason="gate-after")
            for d in range(1, 8):
                eph = gate(nc.tensor, rsems[d], RINC[d] * (nsteps + 1),
                           after=[eph])
            emit_fc(nsteps, ring[nsteps % NSLOT], eph)
            # drain: all outbound transfers complete before program end
            dr = gate(nc.gpsimd, lsem, 16 * 7 * (nsteps + 1), after=[trig])
            if WARMUP_CC:
                # a collective anywhere in the NEFF makes the runtime do a
                # load-time rendezvous + start barrier, aligning core starts
                # (without it, per-core loads serialize and the profiled
                # core's span includes multi-ms start stagger). At the END,
                # after the SWDGE drain, it cannot race remote-DMA traffic
                # (at the start it crashed the exec unit).
                cc = nc.gpsimd.collective_compute(
                    "AllGather", mybir.AluOpType.bypass,
                    replica_groups=[list(range(NCORES))],
                    ins=[warm_in_d], outs=[warm_out_d],
                )
                add_dep_helper(cc.ins, dr.ins, sync=False,
                               reason="cc-after-drain")

    for (ins, sem, val) in pending:
        ins._wait_ge(sem, val)

    if for_sim:
        nc.dce_regs()
        nc.alloc_regs()
        nc.insert_library_loads()
        nc.insert_act_table_loads()
    else:
        nc.compile()
    return nc


_NC_CACHE = None


def _get_nc():
    global _NC_CACHE
    if _NC_CACHE is None:
        _NC_CACHE = build_nc()
    return _NC_CACHE


def _prep_inputs(encoder_output, captions, embed_table, W_ih, W_hh, b_ih, b_hh,
                 W_fc, b_fc, perm=None):
    perm = perm if perm is not None else PERM
    bf = ml_dtypes.bfloat16
    enc = np.asarray(encoder_output, np.float32)
    cap = np.asarray(captions).astype(np.int64)
    emb = np.asarray(embed_table, np.float32)
    W_ih = np.asarray(W_ih, np.float32)
    W_hh = np.asarray(W_hh, np.float32)
    W_fc = np.asarray(W_fc, np.float32)
    bgs = np.asarray(b_ih, np.float32) + np.asarray(b_hh, np.float32)
    b_fc = np.asarray(b_fc, np.float32)

    X = np.empty((T, B, E), np.float32)
    X[0] = enc
    X[1:] = emb[cap[:, : T - 1]].transpose(1, 0, 2)
    xt = np.ascontiguousarray(
        X[1:].reshape(T - 1, B, KX, 128).transpose(0, 3, 2, 1)).astype(bf)

    # step 0 on host, fp32 (h_prev = c_prev = 0)
    gates0 = enc @ W_ih.T + bgs
    i0, f0, g0, o0 = np.split(gates0, 4, axis=-1)
    sig = lambda z: 1.0 / (1.0 + np.exp(-z))
    c0 = sig(i0) * np.tanh(g0)
    h0 = sig(o0) * np.tanh(c0)
    ht0_chunks = np.ascontiguousarray(h0.T.reshape(KH, 128, B))

    common = {"xt": xt}
    in_maps = []
    for ci in range(NCORES):
        prow = perm[ci]
        r = np.r_[ci * 128:(ci + 1) * 128]
        sel = np.concatenate([r, 2048 + r, 1024 + r, 3072 + r])  # [i,g,f,o]
        wih = np.ascontiguousarray(
            W_ih[sel].reshape(4, 128, KX, 128).transpose(3, 2, 0, 1)).astype(bf)
        whh = np.ascontiguousarray(
            W_hh[sel].reshape(4, 128, KH, 128).transpose(3, 2, 0, 1)).astype(bf)
        whh = np.ascontiguousarray(whh[:, prow, :, :])   # slot-permuted
        bgc = np.ascontiguousarray(bgs[sel].reshape(4, 128).T)
        c0j = np.ascontiguousarray(c0[:, ci * 128:(ci + 1) * 128].T)
        ht0 = np.ascontiguousarray(
            ht0_chunks[prow].transpose(1, 0, 2)).astype(bf)  # [128, KH, B]
        vsl = slice(ci * VS, (ci + 1) * VS)
        wfc = np.ascontiguousarray(
            W_fc[vsl].reshape(VS, KH, 128).transpose(2, 1, 0)).astype(bf)
        wfc = np.ascontiguousarray(wfc[:, prow, :])      # slot-permuted
        bfc = np.ascontiguousarray(np.broadcast_to(b_fc[vsl], (128, VS)))
        ids = np.full((128, 16), float(ci), bf)
        in_maps.append({**common, "wih": wih, "whh": whh, "bg": bgc,
                        "c0": c0j, "wfc": wfc, "bfc": bfc, "ht0": ht0,
                        "ids": ids})
    return in_maps


def run_on_device(in_maps, trace=False, **kw):
    nc = _get_nc()
    return run_bass_kernel_spmd(
        nc, in_maps, list(range(NCORES)), trace=trace, **kw)


def _check_topo(res):
    """Returns None if PERM matches, else the discovered table."""
    disc = []
    ok = True
    for ci in range(NCORES):
        topo = np.asarray(res.results[ci]["topo"]).astype(np.float32)
        topo = topo.reshape(128, 8, 16)
        row = [ci]
        for d in range(1, 8):
            v = int(round(float(topo[0, d, 0])))
            row.append(v)
            if v != PERM[ci][d]:
                ok = False
        disc.append(row)
    return None if ok else disc


def _assemble(res):
    shards = [np.asarray(res.results[ci]["logits"]).astype(np.float32)
              for ci in range(NCORES)]
    full = np.concatenate(shards, axis=-1)  # [T, B, V]
    return np.ascontiguousarray(full.transpose(1, 0, 2))  # [B, T, V]


def kernel(encoder_output, captions, embed_table, W_ih, W_hh, b_ih, b_hh,
           W_fc, b_fc):
    args = (encoder_output, captions, embed_table, W_ih, W_hh, b_ih, b_hh,
            W_fc, b_fc)
    in_maps = _prep_inputs(*args)
    res = run_on_device(in_maps)
    disc = _check_topo(res)
    if disc is not None:
        # topology differs from the hardcoded PERM: rebuild + rerun
        in_maps = _prep_inputs(*args, perm=disc)
        res = run_on_device(in_maps)
    return _assemble(res)


# revision 11
# speedup vs baseline: 1.0966x; 1.0966x over previous
"""LSTM caption decoder on 8 TRN2 NeuronCores — hidden-sharded, SBUF->SBUF
remote-DMA h exchange (replaces the ncfw AllGather of the baseline).

Per step, each core broadcasts its h_t chunk [128, B] bf16 to its 7
XOR-partners with 7 single-destination remote_dma_broadcasts: the send to
XOR-distance d lands in the receiver's ring slot d. Receiver slot s therefore
always holds the chunk of its XOR-s partner; the per-core whh/wfc/ht0 inputs
are slot-permuted on the host (PERM table, probe-measured; re-verified at
runtime via a topo output, with automatic in_map rebuild + rerun on
mismatch).

Flow control needs no credits: only the PE reads the ring, the PE queue is
in-order, and trigger_t is gated (via Tile's deferred-RAW edge) on the h_t
vector mul, which transitively implies every ring reader of generation t-2
and earlier has retired. With ring depth 3, a peer's sends for generation
t+3 (which require our h_{t+2} chunk, hence our trigger_{t+2}) can therefore
never overwrite a generation still being read.

Cross-core waits (rsem arrivals, lsem drain) cannot be satisfied in Tile's
single-core scheduling sim, so they ride on placeholder wait instructions
(trivially true at scheduling time) whose real condition is added after the
TileContext exits. A placeholder emitted before its consumers on the same
engine queue is scheduled before them (it is always ready and has an earlier
priority), so it gates the whole queue at exactly the right point.
"""

import sys

if "/opt/trn_rl_repo" not in sys.path:
    sys.path.insert(0, "/opt/trn_rl_repo")

import numpy as np
import ml_dtypes

import concourse.bass as bass
import concourse.bacc as bacc
import concourse.tile as tile
from concourse.tile_rust import add_dep_helper
from concourse import mybir
from concourse.bass_utils import run_bass_kernel_spmd

B = 128
T = 24
E = 512
H = 1024
V = 12000
NCORES = 8
VS = V // NCORES          # 1500 vocab cols per core
KX = E // 128             # 4 contraction chunks for x-part
KH = H // 128             # 8 contraction chunks for h-part
NVC = 3                   # vocab chunks per core (3 x 500)
VC = VS // NVC            # 500
NSLOT = 3                 # ring depth

F32 = mybir.dt.float32
BF16 = mybir.dt.bfloat16
AF = mybir.ActivationFunctionType

# gate col-chunk order [i, g, f, o] -> activation per chunk
_CC_FUNC = [AF.Sigmoid, AF.Tanh, AF.Sigmoid, AF.Sigmoid]

# PERM[i][s] = jax index of core i's XOR-distance-s partner (probe-measured).
PERM = [
    [0, 1, 2, 3, 6, 7, 4, 5],
    [1, 0, 3, 2, 7, 6, 5, 4],
    [2, 3, 0, 1, 4, 5, 6, 7],
    [3, 2, 1, 0, 5, 4, 7, 6],
    [4, 5, 6, 7, 2, 3, 0, 1],
    [5, 4, 7, 6, 3, 2, 1, 0],
    [6, 7, 4, 5, 0, 1, 2, 3],
    [7, 6, 5, 4, 1, 0, 3, 2],
]

NSTEPS = T - 1
# An ncfw collective at program START crashes the exec unit when the program
# also uses SWDGE remote_dma (this was the earlier session's "repeated
# broadcasts crash"). But a collective-free NEFF loses the runtime's
# load-time rendezvous, so core starts stagger by milliseconds, which lands
# in the profiled span. Solution: one collective at program END, after the
# SWDGE drain.
WARMUP_CC = True


def build_nc(for_sim: bool = False, nsteps: int = NSTEPS):
    nc = bacc.Bacc("TRN2", target_bir_lowering=False, debug=False,
                   num_devices=NCORES)

    wih_d = nc.dram_tensor("wih", [128, KX, 4, 128], BF16, kind="ExternalInput").ap()
    whh_d = nc.dram_tensor("whh", [128, KH, 4, 128], BF16, kind="ExternalInput").ap()
    bg_d = nc.dram_tensor("bg", [128, 4], F32, kind="ExternalInput").ap()
    wfc_d = nc.dram_tensor("wfc", [128, KH, VS], BF16, kind="ExternalInput").ap()
    bfc_d = nc.dram_tensor("bfc", [128, VS], F32, kind="ExternalInput").ap()
    xt_d = nc.dram_tensor("xt", [T - 1, 128, KX, B], BF16, kind="ExternalInput").ap()
    ht0_d = nc.dram_tensor("ht0", [128, KH, B], BF16, kind="ExternalInput").ap()
    c0_d = nc.dram_tensor("c0", [128, B], F32, kind="ExternalInput").ap()
    ids_d = nc.dram_tensor("ids", [128, 16], BF16, kind="ExternalInput").ap()
    out_d = nc.dram_tensor("logits", [T, B, VS], BF16, kind="ExternalOutput").ap()
    topo_d = nc.dram_tensor("topo", [128, 8, 16], BF16, kind="ExternalOutput").ap()
    warm_in_d = nc.dram_tensor("warm_in", [1, 4], BF16, kind="Internal").ap()
    warm_out_d = nc.dram_tensor("warm_out", [KH, 4], BF16, kind="Internal",
                                addr_space="Shared").ap()

    # (placeholder wait instruction, sem, value): real condition added after
    # Tile scheduling — the scheduler's single-core sim can't satisfy
    # cross-core sems and would deadlock.
    pending = []
    all_preps = []

    import contextlib
    raw = contextlib.ExitStack()
    # ring + topo receive buffers are RAW sbuf tensors, invisible to Tile's
    # dependency tracking: remote writes + the local slot-0 h store are
    # ordered purely by the rsem/hsem protocol. (Tile's conservative WAR
    # edges on tracked tiles serialized the broadcast preps behind late PE
    # ticks, delaying straggler transfers by a whole step.)
    ring = [raw.enter_context(
        nc.sbuf_tensor(f"ring{r}", [128, KH, B], BF16)) for r in range(NSLOT)]
    topo_rx = raw.enter_context(nc.sbuf_tensor("topo_rx", [128, 8, 16], BF16))

    with raw, tile.TileContext(nc) as tc:
        with (
            tc.tile_pool(name="weights", bufs=1) as wpool,
            tc.tile_pool(name="xin", bufs=3) as xpool,
            tc.tile_pool(name="gact", bufs=2) as gpool,
            tc.tile_pool(name="state", bufs=1) as spool,
            tc.tile_pool(name="hbuf", bufs=1) as hpool,
            tc.tile_pool(name="lout", bufs=4) as lpool,
            tc.tile_pool(name="pg", bufs=4, space="PSUM") as pgpool,
            tc.tile_pool(name="pf", bufs=3, space="PSUM") as pfpool,
        ):
            wih = wpool.tile([128, KX, 4, 128], BF16)
            whh = wpool.tile([128, KH, 4, 128], BF16)
            bg = wpool.tile([128, 4], F32)
            wfc = wpool.tile([128, KH, VS], BF16)
            bfc = wpool.tile([128, VS], F32)
            c = spool.tile([128, B], F32)
            tanh_c = spool.tile([128, B], F32)
            ig = spool.tile([128, B], F32)
            ids_t = hpool.tile([128, 16], BF16)

            rsems = [nc.alloc_semaphore(f"rsem{d}") for d in range(8)]
            # arrivals per round: same-die slots (1-3) +16, cross-die +8
            RINC = [0, 16, 16, 16, 8, 8, 8, 8]
            lsem = nc.alloc_semaphore("lsem")     # local send completions
            hsem = nc.alloc_semaphore("hsem")     # h-ready chain (hm_t -> t)
            phsem = nc.alloc_semaphore("phsem")   # placeholder-only, stays 0

            def gate(eng, sem, val, after=()):
                """Placeholder wait on `eng`'s queue; the real cross-core
                condition (sem >= val) is added after Tile scheduling.
                `after`: instructions the gate must schedule behind (no-sync
                edges); callers add gate->consumer edges via before()."""
                ph = eng.wait_ge(phsem, 0)
                for a in after:
                    add_dep_helper(ph.ins, a.ins, sync=False,
                                   reason="gate-after")
                pending.append((ph, sem, val))
                return ph

            def before(ph, consumers):
                """Pin consumers behind the gate in the schedule."""
                if ph is None:
                    return
                for cns in consumers:
                    add_dep_helper(cns.ins, ph.ins, sync=False,
                                   reason="gate-before")


            # prologue loads
            ht0_t = hpool.tile([128, KH, B], BF16, name="ht0t")
            nc.sync.dma_start(ht0_t[:], ht0_d[:])
            nc.sync.dma_start(c[:], c0_d[:])
            xt1 = xpool.tile([128, KX, B], BF16, tag="xt")
            nc.sync.dma_start(xt1[:], xt_d[0])
            nc.sync.dma_start(wih[:], wih_d[:])
            nc.sync.dma_start(bg[:], bg_d[:])
            nc.sync.dma_start(whh[:], whh_d[:])
            nc.sync.dma_start(ids_t[:], ids_d[:])
            nc.sync.dma_start(topo_rx[:, 0, :], ids_d[:, 0:16])
            for v in range(NVC):
                vsl = slice(v * VC, (v + 1) * VC)
                eng = nc.scalar if v == 1 else nc.sync
                eng.dma_start(wfc[:, :, vsl], wfc_d[:, :, vsl])
            nc.scalar.dma_start(bfc[:], bfc_d[:])

            def send_group(src_ap, dst_tile):
                """7 single-dest broadcasts src->slot d + trigger. The one
                destination is repeated across slots so the payload stripes
                over all eligible lanes (16 same-die, 8 D2D for cross-die)
                instead of 2 — the frame is 1024 descriptors either way and
                payload descriptors drain much faster than dummies.
                (Genuinely multi-dest data broadcasts crash the exec unit
                on this runtime.) Each distance gets its own arrival sem so
                receivers can start consuming a slot as soon as it lands."""
                for d in range(1, 8):
                    if d < 4:
                        rd = [(0, d)] * 8
                    else:
                        rd = [None] * 4 + [(0, d)] * 4
                    all_preps.append(nc.gpsimd.remote_dma_broadcast(
                        dst_tile[:, d, :], src_ap, rsems[d], lsem, rdests=rd))
                return nc.gpsimd.trigger_dma(count=None)

            # topo round: exchange jax ids (verifies PERM at runtime)
            ttrig = send_group(ids_t[:], topo_rx)
            tph = ttrig
            for d in range(1, 8):
                tph = gate(nc.gpsimd, rsems[d], RINC[d], after=[tph])
            before(tph, [nc.gpsimd.dma_start(topo_d[:], topo_rx[:])])

            def emit_fc(t, rp, ph):
                """FC over ring rp (gen t), pinned behind gate `ph` (the
                same-ring-generation arrival gate)."""
                firsts, last = [], None
                for v in range(NVC):
                    vsl = slice(v * VC, (v + 1) * VC)
                    pf = pfpool.tile([B, VC], F32, tag="pf")
                    for k in range(KH):
                        mm = nc.tensor.matmul(
                            pf[:], rp[:, k, :], wfc[:, k, vsl],
                            start=(k == 0), stop=(k == KH - 1),
                        )
                        if k == 0:
                            firsts.append(mm)
                        last = mm
                    lo = lpool.tile([B, VC], BF16, tag="lo")
                    nc.vector.tensor_add(lo[:], pf[:], bfc[:, vsl])
                    nc.scalar.dma_start(out_d[t, :, vsl], lo[:])
                before(ph, firsts)
                return last

            for t in range(1, nsteps + 1):
                if t == 1:
                    xt = xt1
                else:
                    xt = xpool.tile([128, KX, B], BF16, tag="xt")
                    nc.scalar.dma_start(xt[:], xt_d[t - 1])
                # gen 0 lives in the tracked ht0 tile (Tile orders its DMA);
                # gens >= 1 live in the raw rings
                rp = ht0_t if t == 1 else ring[(t - 1) % NSLOT]
                rc = ring[t % NSLOT]         # will hold h_t

                # ---- gates (transposed): x-parts first, then h-parts ----
                pgs = []
                xp_last = []
                for cc in range(4):
                    pg = pgpool.tile([128, 512], F32, tag="pg")
                    pgs.append(pg)
                    for k in range(KX):
                        mm = nc.tensor.matmul(
                            pg[:, 0:B], wih[:, k, cc, :], xt[:, k, :],
                            start=(k == 0), stop=False,
                        )
                    xp_last.append(mm)
                # slot-major h-part matmuls: slot s's 4 gate-col MMs run as
                # soon as its chunk lands (per-slot gates chained on the PE
                # queue). Slot 0 (own chunk, t>=2) gates on the hsem chain.
                ph = None
                if t >= 2:
                    ph = nc.tensor.wait_ge(hsem, t - 1)
                    for a_ in xp_last:
                        add_dep_helper(ph.ins, a_.ins, sync=False,
                                       reason="gate-after")
                for s in range(KH):
                    if t >= 2 and s >= 1:
                        ph = gate(nc.tensor, rsems[s], RINC[s] * t,
                                  after=[ph] if ph is not None else [])
                    for cc in range(4):
                        mm = nc.tensor.matmul(
                            pgs[cc][:, 0:B], whh[:, s, cc, :], rp[:, s, :],
                            start=False, stop=(s == KH - 1),
                        )
                        before(ph, [mm])
                gact = []
                for cc in range(4):
                    a = gpool.tile([128, B], F32, tag=f"g{cc}")
                    nc.scalar.activation(a[:], pgs[cc][:, 0:B], _CC_FUNC[cc],
                                         bias=bg[:, cc:cc + 1])
                    gact.append(a)
                    if cc == 1:
                        nc.vector.tensor_mul(ig[:], gact[0][:], gact[1][:])
                    elif cc == 2:
                        nc.vector.tensor_mul(c[:], c[:], gact[2][:])
                        nc.vector.tensor_add(c[:], c[:], ig[:])
                        nc.scalar.activation(tanh_c[:], c[:], AF.Tanh)
                    elif cc == 3:
                        hm = nc.vector.tensor_mul(rc[:, 0, :], gact[3][:],
                                                  tanh_c[:])
                        # dedicated sem op (DVE TT has no free update slot);
                        # in-order DVE queue + no-sync edge pins it after hm
                        hs = nc.vector.sem_inc(hsem, 1)
                        add_dep_helper(hs.ins, hm.ins, sync=False,
                                       reason="hsem-after-hm")

                # ---- broadcast h_t (trigger waits h_t via hsem) ----
                trig = send_group(rc[:, 0, :], rc)
                trig._wait_ge(hsem, t)

                # ---- FC of previous step (overlaps the broadcast) ----
                fc_last = emit_fc(t - 1, rp, ph)


            eph = nc.tensor.wait_ge(hsem, nsteps)
            add_dep_helper(eph.ins, fc_last.ins, sync=False,
                           reason="gate-after")
            for d in range(1, 8):
                eph = gate(nc.tensor, rsems[d], RINC[d] * (nsteps + 1),
                           after=[eph])
            emit_fc(nsteps, ring[nsteps % NSLOT], eph)
            # drain: all outbound transfers complete before program end
            dr = gate(nc.gpsimd, lsem, 16 * 7 * (nsteps + 1), after=[trig])
            if WARMUP_CC:
                # a collective anywhere in the NEFF makes the runtime do a
                # load-time rendezvous + start barrier, aligning core starts
                # (without it, per-core loads serialize and the profiled
                # core's span includes multi-ms start stagger). At the END,
                # after the SWDGE drain, it cannot race remote-DMA traffic
                # (at the start it crashed the exec unit).
                cc = nc.gpsimd.collective_compute(
                    "AllGather", mybir.AluOpType.bypass,
                    replica_groups=[list(range(NCORES))],
                    ins=[warm_in_d], outs=[warm_out_d],
                )
                add_dep_helper(cc.ins, dr.ins, sync=False,
                               reason="cc-after-drain")

    for (ins, sem, val) in pending:
        ins._wait_ge(sem, val)

    # Strip PE/DVE sync deps from the broadcast preps. These are bass's
    # conservative WAR/RAW edges on the ring slots; at compile they become
    # per-prep PE-tick waits that couple descriptor generation to current
    # FC matmul progress (~10us/step of pure serialization). The rsem/hsem
    # protocol provides the same guarantees: a slot is only rewritten three
    # generations later, by which time the arrival chain proves all its
    # readers retired, and the h_t data read happens at trigger time, which
    # waits hsem >= t.
    strip = {mybir.EngineType.PE, mybir.EngineType.DVE}
    for p in all_preps:
        ins = p.ins
        for dn in list(ins.sync_dependency_names()):
            o = nc.inst_map.get(dn)
            if o is not None and getattr(o, 'engine', None) in strip:
                ins.remove_dependency(dn)

    if for_sim:
        nc.dce_regs()
        nc.alloc_regs()
        nc.insert_library_loads()
        nc.insert_act_table_loads()
    else:
        nc.compile()
    return nc


_NC_CACHE = None


def _get_nc():
    global _NC_CACHE
    if _NC_CACHE is None:
        _NC_CACHE = build_nc()
    return _NC_CACHE


def _prep_inputs(encoder_output, captions, embed_table, W_ih, W_hh, b_ih, b_hh,
                 W_fc, b_fc, perm=None):
    perm = perm if perm is not None else PERM
    bf = ml_dtypes.bfloat16
    enc = np.asarray(encoder_output, np.float32)
    cap = np.asarray(captions).astype(np.int64)
    emb = np.asarray(embed_table, np.float32)
    W_ih = np.asarray(W_ih, np.float32)
    W_hh = np.asarray(W_hh, np.float32)
    W_fc = np.asarray(W_fc, np.float32)
    bgs = np.asarray(b_ih, np.float32) + np.asarray(b_hh, np.float32)
    b_fc = np.asarray(b_fc, np.float32)

    X = np.empty((T, B, E), np.float32)
    X[0] = enc
    X[1:] = emb[cap[:, : T - 1]].transpose(1, 0, 2)
    xt = np.ascontiguousarray(
        X[1:].reshape(T - 1, B, KX, 128).transpose(0, 3, 2, 1)).astype(bf)

    # step 0 on host, fp32 (h_prev = c_prev = 0)
    gates0 = enc @ W_ih.T + bgs
    i0, f0, g0, o0 = np.split(gates0, 4, axis=-1)
    sig = lambda z: 1.0 / (1.0 + np.exp(-z))
    c0 = sig(i0) * np.tanh(g0)
    h0 = sig(o0) * np.tanh(c0)
    ht0_chunks = np.ascontiguousarray(h0.T.reshape(KH, 128, B))

    common = {"xt": xt}
    in_maps = []
    for ci in range(NCORES):
        prow = perm[ci]
        r = np.r_[ci * 128:(ci + 1) * 128]
        sel = np.concatenate([r, 2048 + r, 1024 + r, 3072 + r])  # [i,g,f,o]
        wih = np.ascontiguousarray(
            W_ih[sel].reshape(4, 128, KX, 128).transpose(3, 2, 0, 1)).astype(bf)
        whh = np.ascontiguousarray(
            W_hh[sel].reshape(4, 128, KH, 128).transpose(3, 2, 0, 1)).astype(bf)
        whh = np.ascontiguousarray(whh[:, prow, :, :])   # slot-permuted
        bgc = np.ascontiguousarray(bgs[sel].reshape(4, 128).T)
        c0j = np.ascontiguousarray(c0[:, ci * 128:(ci + 1) * 128].T)
        ht0 = np.ascontiguousarray(
            ht0_chunks[prow].transpose(1, 0, 2)).astype(bf)  # [128, KH, B]
        vsl = slice(ci * VS, (ci + 1) * VS)
        wfc = np.ascontiguousarray(
            W_fc[vsl].reshape(VS, KH, 128).transpose(2, 1, 0)).astype(bf)
        wfc = np.ascontiguousarray(wfc[:, prow, :])      # slot-permuted
        bfc = np.ascontiguousarray(np.broadcast_to(b_fc[vsl], (128, VS)))
        ids = np.full((128, 16), float(ci), bf)
        in_maps.append({**common, "wih": wih, "whh": whh, "bg": bgc,
                        "c0": c0j, "wfc": wfc, "bfc": bfc, "ht0": ht0,
                        "ids": ids})
    return in_maps


def run_on_device(in_maps, trace=False, **kw):
    nc = _get_nc()
    return run_bass_kernel_spmd(
        nc, in_maps, list(range(NCORES)), trace=trace, **kw)


def _check_topo(res):
    """Returns None if PERM matches, else the discovered table."""
    disc = []
    ok = True
    for ci in range(NCORES):
        topo = np.asarray(res.results[ci]["topo"]).astype(np.float32)
        topo = topo.reshape(128, 8, 16)
        row = [ci]
        for d in range(1, 8):
            v = int(round(float(topo[0, d, 0])))
            row.append(v)
            if v != PERM[ci][d]:
                ok = False
        disc.append(row)
    return None if ok else disc


def _assemble(res):
    shards = [np.asarray(res.results[ci]["logits"]).astype(np.float32)
              for ci in range(NCORES)]
    full = np.concatenate(shards, axis=-1)  # [T, B, V]
    return np.ascontiguousarray(full.transpose(1, 0, 2))  # [B, T, V]


def kernel(encoder_output, captions, embed_table, W_ih, W_hh, b_ih, b_hh,
           W_fc, b_fc):
    args = (encoder_output, captions, embed_table, W_ih, W_hh, b_ih, b_hh,
            W_fc, b_fc)
    in_maps = _prep_inputs(*args)
    res = run_on_device(in_maps)
    disc = _check_topo(res)
    if disc is not None:
        # topology differs from the hardcoded PERM: rebuild + rerun
        in_maps = _prep_inputs(*args, perm=disc)
        res = run_on_device(in_maps)
    return _assemble(res)
